# revision 1
# baseline (speedup 1.0000x reference)
"""Multi-head attention (B=2, S=2048, H=1024, 16 heads x 64) on 8 NeuronCores.

Sharding: tensor-parallel over heads x data-parallel over batch.
Core c handles batch (c // 4) and heads [4*(c%4), 4*(c%4)+4).
Each core computes its 4 heads' QKV projections, attention, and the partial
output projection ctx_h @ Wo_h; the host sums the 4 partials per batch.

All matmuls run as float32r (full-rate fp32 mode on the PE array).
Softmax skips max-subtraction (scores are O(+-10) here; exp is exact to 2ULP)
and gets its denominator for free from an appended ones-column on V.

Schedule: QKV projections for the second head-pair are interleaved into the
first head-pair's attention loop (which is otherwise exp-bound on ScalarE),
keeping the PE busy. exp runs as [128,1024] ops spanning two PSUM banks to
amortize the ScalarE access-latency overhead.
"""
import numpy as np

import concourse.bass as bass
import concourse.tile as tile
from concourse import bacc, mybir
from concourse.bass_utils import run_bass_kernel_spmd
from concourse.masks import make_identity

F32 = mybir.dt.float32
F32R = mybir.dt.float32r

H, NH, HD = 1024, 16, 64
B, S = 2, 2048
P = 128
NCORES = 8
NHL = 4          # heads per core
DQ = NHL * HD    # 256 projection cols per core
NHT = H // P     # 8 h-tiles
NST = S // P     # 16 s-tiles (also t-tiles)
SB = 512         # matmul free-dim block
SS = 1024        # exp super-block (2 PSUM banks)
NSB = S // SB    # 4
NSS = S // SS    # 2


def build_program(repeat=1, ct=None, lite_exp=False):
    CT = F32R if ct is None else ct
    XV = F32 if CT == F32R else CT
    nc = bacc.Bacc("TRN2", target_bir_lowering=False, debug=False,
                   num_devices=NCORES)
    if CT != F32R:
        _lp = nc.allow_low_precision(reason="bf16 timing variant")
        _lp.__enter__()

    xt_d = nc.dram_tensor("xt", [H, S], CT, kind="ExternalInput").ap()
    wq_d = nc.dram_tensor("wq", [H, DQ], CT, kind="ExternalInput").ap()
    wk_d = nc.dram_tensor("wk", [H, DQ], CT, kind="ExternalInput").ap()
    wv_d = nc.dram_tensor("wv", [H, DQ], CT, kind="ExternalInput").ap()
    wo_d = nc.dram_tensor("wo", [DQ, H], CT, kind="ExternalInput").ap()
    bq_d = nc.dram_tensor("bq", [P, 2], F32, kind="ExternalInput").ap()
    bk_d = nc.dram_tensor("bk", [P, 2], F32, kind="ExternalInput").ap()
    bv_d = nc.dram_tensor("bv", [P, 2], F32, kind="ExternalInput").ap()
    mb_d = nc.dram_tensor("maskb", [P, NST], F32, kind="ExternalInput").ap()
    part_d = nc.dram_tensor("part", [S, H], F32, kind="ExternalOutput").ap()

    scr_den = nc.dram_tensor("scr_den", [NHL, S], XV).ap()
    scr_rec = nc.dram_tensor("scr_rec", [NHL, S], XV).ap()

    with tile.TileContext(nc) as tc:
        with tc.tile_pool(name="big", bufs=1) as big, \
             tc.tile_pool(name="consts", bufs=1) as consts, \
             tc.tile_pool(name="epool", bufs=3) as epool, \
             tc.tile_pool(name="ctxpool", bufs=4) as ctxpool, \
             tc.tile_pool(name="vtpool", bufs=16) as vtpool, \
             tc.tile_pool(name="bcpool", bufs=1) as bcpool, \
             tc.tile_pool(name="opool", bufs=4) as opool, \
             tc.tile_pool(name="dpool", bufs=2) as dpool, \
             tc.tile_pool(name="ps_sc", bufs=2, space="PSUM") as ps_sc, \
             tc.tile_pool(name="ps_ctx", bufs=1, space="PSUM") as ps_ctx, \
             tc.tile_pool(name="ps_mm", bufs=2, space="PSUM") as ps_mm:

            for _it in range(repeat):
                # ---- input loads ----
                # DMA emission order tracks consumption order: wq, then X
                # s-blocks (first projection group only waits ~3MB), weights
                # for K/V between the later X blocks.
                xt_sb = big.tile([P, NHT, S], CT, tag="xt", name="xt_sb")
                xt_r = xt_d.rearrange("(n p) s -> n p s", p=P)
                wq_sb = consts.tile([P, NHT, DQ], CT, tag="wq", name="wq_sb")
                wk_sb = consts.tile([P, NHT, DQ], CT, tag="wk", name="wk_sb")
                wv_sb = consts.tile([P, NHT, DQ], CT, tag="wv", name="wv_sb")

                def load_x_block(sb_i):
                    for ht in range(NHT):
                        nc.sync.dma_start(
                            out=xt_sb[:, ht, sb_i * SB:(sb_i + 1) * SB],
                            in_=xt_r[ht, :, sb_i * SB:(sb_i + 1) * SB])

                def load_w(w_sb, w_d):
                    nc.sync.dma_start(
                        out=w_sb, in_=w_d.rearrange("(n p) d -> p n d", p=P))

                load_w(wq_sb, wq_d)
                load_x_block(0)
                load_w(wk_sb, wk_d)
                load_w(wv_sb, wv_d)
                load_x_block(1)
                load_x_block(2)
                load_x_block(3)
                # Wo rows for head h at partitions 0..63, index h. Shares the
                # wq slot (wq is dead once the Q projection finishes).
                wo_sb = consts.tile([HD, NHL, H], CT, tag="wq", name="wo_sb")
                nc.sync.dma_start(
                    out=wo_sb, in_=wo_d.rearrange("(h p) o -> p h o", p=HD))

                bq_sb = consts.tile([P, 2], F32, tag="bq", name="bq_sb")
                bk_sb = consts.tile([P, 2], F32, tag="bk", name="bk_sb")
                bv_sb = consts.tile([P, 2], F32, tag="bv", name="bv_sb")
                for b_sb, b_d in ((bq_sb, bq_d), (bk_sb, bk_d), (bv_sb, bv_d)):
                    nc.sync.dma_start(out=b_sb, in_=b_d)
                mb_sb = consts.tile([P, NST], F32, tag="mb", name="mb_sb")
                nc.sync.dma_start(out=mb_sb, in_=mb_d)

                ident = consts.tile([P, P], F32, tag="ident", name="ident")
                make_identity(nc, ident)

                # V in [t, head, dv] layout + ones column (denominator trick).
                # walrus rejects memset on f32r; broadcast-copy 1.0 instead.
                vaug = big.tile([P, NST, NHL, HD + 1], CT, tag="vaug",
                                name="vaug")
                one = nc.const_aps.aps[(F32, 1.0)]
                ones_src = bass.AP(tensor=one.tensor, offset=one.offset,
                                   ap=[one.ap[0], [0, NST], [0, NHL], [0, 1]])
                nc.vector.tensor_copy(vaug[:, :, :, HD:HD + 1], ones_src)

                qT = big.tile([P, 2, S], CT, tag="qT", name="qT")
                kT = big.tile([P, 2, S], CT, tag="kT", name="kT")

                # ---- projection task list for one head pair (dqt) ----
                # Each task emits one PSUM accumulation group (8 matmuls) +
                # its drain, or a batch of V transposes. Tasks for pair 1 are
                # drip-fed into pair 0's attention loop as PE filler.
                def make_proj_tasks(dqt):
                    tasks = []

                    def qk_group(w_sb, b_sb, out_sb, sb_i):
                        def t():
                            acc = ps_mm.tile([P, SB], F32, tag="mm512",
                                             name=f"acc_{dqt}_{sb_i}")
                            for ht in range(NHT):
                                nc.tensor.matmul(
                                    acc,
                                    w_sb[:, ht, dqt * P:(dqt + 1) * P],
                                    xt_sb[:, ht, sb_i * SB:(sb_i + 1) * SB],
                                    start=(ht == 0), stop=(ht == NHT - 1))
                            nc.vector.tensor_scalar_add(
                                out_sb[:, dqt, sb_i * SB:(sb_i + 1) * SB],
                                acc, b_sb[:, dqt:dqt + 1])
                        return t

                    def v_group(sb_i, chunks_out):
                        def t():
                            acc = ps_mm.tile([P, SB], F32, tag="mm512",
                                             name=f"vacc_{dqt}_{sb_i}")
                            for ht in range(NHT):
                                nc.tensor.matmul(
                                    acc,
                                    wv_sb[:, ht, dqt * P:(dqt + 1) * P],
                                    xt_sb[:, ht, sb_i * SB:(sb_i + 1) * SB],
                                    start=(ht == 0), stop=(ht == NHT - 1))
                            for k in range(SB // P):
                                st = sb_i * (SB // P) + k
                                ch = vtpool.tile([P, P], F32, tag="vt",
                                                 name=f"vt_{dqt}_{st}")
                                nc.vector.tensor_scalar_add(
                                    ch, acc[:, k * P:(k + 1) * P],
                                    bv_sb[:, dqt:dqt + 1])
                                chunks_out.append((st, ch))
                        return t

                    def tr_one(chunks, idx):
                        def t():
                            st, ch = chunks[idx]
                            tr = ps_mm.tile([P, P], F32, tag="mm512",
                                            name=f"tr_{dqt}_{st}")
                            nc.tensor.transpose(tr, ch, ident)
                            nc.vector.tensor_copy(
                                vaug[:, st, 2 * dqt, 0:HD], tr[:, 0:HD])
                            nc.vector.tensor_copy(
                                vaug[:, st, 2 * dqt + 1, 0:HD], tr[:, HD:P])
                        return t

                    vchunks = []
                    for sb_i in range(NSB):
                        tasks.append(qk_group(wq_sb, bq_sb, qT, sb_i))
                        tasks.append(qk_group(wk_sb, bk_sb, kT, sb_i))
                        tasks.append(v_group(sb_i, vchunks))
                    trs = [tr_one(vchunks, i) for i in range(NST)]
                    return tasks, trs

                # ---- attention for one head; `filler` drips PE tasks ----
                ctxU = [None] * NHL

                def attention(h, filler, rate=4):
                    base = HD * (h % 2)
                    dvt = h // 2
                    cu = ctxpool.tile([HD + 1, S], CT, tag="ctxU",
                                      name=f"ctxU_{h}")
                    ctxU[h] = cu
                    step = 0
                    for ssb in range(NSS):
                        acc = ps_ctx.tile([HD + 1, SS], F32, tag="ctxps",
                                          name=f"ctx_{h}_{ssb}")
                        prev_e = None
                        for tt in range(NST + 1):
                            if filler and step % rate == 0:
                                filler.pop(0)()
                            if tt < NST:
                                sc = ps_sc.tile([P, SS], F32, tag="sc",
                                                name=f"sc_{h}_{ssb}_{tt}")
                                for half in range(2):
                                    sb_i = 2 * ssb + half
                                    nc.tensor.matmul(
                                        sc[:, half * SB:(half + 1) * SB],
                                        kT[base:base + HD, dvt,
                                           tt * P:(tt + 1) * P],
                                        qT[base:base + HD, dvt,
                                           sb_i * SB:(sb_i + 1) * SB],
                                        start=True, stop=True)
                                if lite_exp and tt > 0:
                                    e = prev_e
                                else:
                                    e = epool.tile([P, SS], CT, tag="e",
                                                   name=f"e_{h}_{ssb}_{tt}")
                                    nc.scalar.activation(
                                        out=e, in_=sc,
                                        func=mybir.ActivationFunctionType.Exp,
                                        bias=mb_sb[:, tt:tt + 1], scale=1.0)
                            if tt > 0:
                                for half in range(2):
                                    nc.tensor.matmul(
                                        acc[:, half * SB:(half + 1) * SB],
                                        vaug[:, tt - 1, h, :],
                                        prev_e[:, half * SB:(half + 1) * SB],
                                        start=(tt == 1), stop=(tt == NST))
                            prev_e = e
                            step += 1
                        for half in range(2):
                            sb_i = 2 * ssb + half
                            nc.vector.tensor_copy(
                                cu[:, sb_i * SB:(sb_i + 1) * SB],
                                acc[:, half * SB:(half + 1) * SB])
                    # denominator -> reciprocal -> per-s broadcast scale
                    nc.sync.dma_start(out=scr_den[h],
                                      in_=cu[HD:HD + 1, :].bitcast(XV))
                    den = dpool.tile([P, NST], XV, tag="den",
                                     name=f"den_{h}")
                    nc.sync.dma_start(
                        out=den,
                        in_=scr_den[h].rearrange("(p k) -> p k", p=P))
                    rec = dpool.tile([P, NST], XV, tag="rec",
                                     name=f"rec_{h}")
                    nc.vector.reciprocal(rec, den)
                    nc.sync.dma_start(
                        out=scr_rec[h].rearrange("(p k) -> p k", p=P),
                        in_=rec)
                    for sb_i in range(NSB):
                        row = scr_rec[h, sb_i * SB:(sb_i + 1) * SB]
                        bcast_in = bass.AP(tensor=row.tensor,
                                           offset=row.offset,
                                           ap=[[0, HD]] + row.ap)
                        bc = bcpool.tile([HD, SB], XV, tag="bc",
                                         name=f"bc_{h}_{sb_i}")
                        nc.sync.dma_start(out=bc, in_=bcast_in)
                        nc.vector.tensor_mul(
                            cu[0:HD, sb_i * SB:(sb_i + 1) * SB],
                            cu[0:HD, sb_i * SB:(sb_i + 1) * SB], bc)

                # ---- schedule ----
                p0_tasks, p0_trs = make_proj_tasks(0)
                # pair 0 inline (ACT idle); transposes one V-group behind
                sched0 = (p0_tasks[0:6] + p0_trs[0:4] + p0_tasks[6:9]
                          + p0_trs[4:8] + p0_tasks[9:12] + p0_trs[8:16])
                for t in sched0:
                    t()
                p1_tasks, p1_trs = make_proj_tasks(1)
                attention(0, p1_tasks, rate=6)
                attention(1, p1_tasks, rate=6)
                for t in p1_tasks:             # leftovers, if any
                    t()
                attention(2, p1_trs, rate=1)
                attention(3, None)

                # ---- output projection (partial over local heads) ----
                for st in range(NST):
                    # [128,1024] super-tiles; alternate between the (now idle)
                    # scores and ctx pools so three stiles are in flight
                    if st % 3 == 2:
                        po = ps_ctx.tile([P, H], F32, tag="ctxps",
                                         name=f"po_{st}")
                    else:
                        po = ps_sc.tile([P, H], F32, tag="sc", name=f"po_{st}")
                    for j in range(2):
                        for h in range(NHL):
                            nc.tensor.matmul(
                                po[:, j * SB:(j + 1) * SB],
                                ctxU[h][0:HD, st * P:(st + 1) * P],
                                wo_sb[:, h, j * SB:(j + 1) * SB],
                                start=(h == 0), stop=(h == NHL - 1))
                    for j in range(2):
                        o_half = opool.tile([P, SB], F32, tag="o",
                                            name=f"o_{st}_{j}")
                        nc.vector.tensor_copy(
                            o_half, po[:, j * SB:(j + 1) * SB])
                        nc.sync.dma_start(
                            out=part_d[st * P:(st + 1) * P,
                                       j * SB:(j + 1) * SB],
                            in_=o_half)

    nc.compile()
    return nc


_CACHE = {}


def _get_program(repeat=1, ct=None, lite_exp=False):
    key = (repeat, str(ct), lite_exp)
    if key not in _CACHE:
        _CACHE[key] = build_program(repeat, ct, lite_exp)
    return _CACHE[key]


def _make_in_maps(inputs):
    X = np.asarray(inputs["X"], dtype=np.float32)
    mask = np.asarray(inputs["mask"], dtype=np.float32)
    Wq = np.asarray(inputs["Wq"], dtype=np.float32)
    Wk = np.asarray(inputs["Wk"], dtype=np.float32)
    Wv = np.asarray(inputs["Wv"], dtype=np.float32)
    Wo = np.asarray(inputs["Wo"], dtype=np.float32)
    bq = np.asarray(inputs["bq"], dtype=np.float32)
    bk = np.asarray(inputs["bk"], dtype=np.float32)
    bv = np.asarray(inputs["bv"], dtype=np.float32)

    scale = np.float32(1.0 / np.sqrt(HD))
    in_maps = []
    xts = [np.ascontiguousarray(X[b].T) for b in range(B)]
    maskbs = [np.ascontiguousarray(-1e6 * (1.0 - mask[b])) for b in range(B)]
    for c in range(NCORES):
        b = c // 4
        g = c % 4
        cols = slice(g * DQ, (g + 1) * DQ)
        in_maps.append({
            "xt": xts[b],
            "wq": np.ascontiguousarray(Wq[:, cols] * scale),
            "wk": np.ascontiguousarray(Wk[:, cols]),
            "wv": np.ascontiguousarray(Wv[:, cols]),
            "wo": np.ascontiguousarray(Wo[cols, :]),
            "bq": np.ascontiguousarray((bq[cols] * scale).reshape(2, 128).T),
            "bk": np.ascontiguousarray(bk[cols].reshape(2, 128).T),
            "bv": np.ascontiguousarray(bv[cols].reshape(2, 128).T),
            "maskb": np.ascontiguousarray(maskbs[b].reshape(16, 128).T),
        })
    return in_maps


def kernel(X, mask, Wq, bq, Wk, bk, Wv, bv, Wo, bo):
    bo = np.asarray(bo, dtype=np.float32)
    nc = _get_program()
    in_maps = _make_in_maps(dict(X=X, mask=mask, Wq=Wq, bq=bq, Wk=Wk, bk=bk,
                                 Wv=Wv, bv=bv, Wo=Wo, bo=bo))
    res = run_bass_kernel_spmd(nc, in_maps, list(range(NCORES))).results
    out = np.zeros((B, S, H), dtype=np.float32)
    for c in range(NCORES):
        out[c // 4] += res[c]["part"]
    out += bo
    return out



# revision 57
# speedup vs baseline: 18.8949x; 18.8949x over previous
"""Multi-head attention (B=2, S=2048, H=1024, 16 heads x 64) on 8 NeuronCores.

Sharding: tensor-parallel over heads x data-parallel over batch.
Core c handles batch (c // 4) and heads [4*(c%4), 4*(c%4)+4).
Each core computes its 4 heads' QKV projections, attention, and the partial
output projection ctx_h @ Wo_h; the host sums the 4 partials per batch.

The datapath is fp16 (noise ~5e-4; fp8 was tried and its ~2.5%/stage
quantization noise transfers 1:1 through the softmax-weighted mean, far
over the accuracy budget). fp16 matmuls run at the same 1 cycle/row as
fp32r but with half the SBUF/DMA traffic. Structural savings vs the fp32
baseline:
 - V is computed directly in [t, dv] layout by making X the stationary
   matmul operand, eliminating all PE transposes and their drains.
 - The output projection packs the two heads of a pair on the contraction
   dim (K=128 instead of 64), halving its PE time. It runs in two passes:
   head-pair 0 as PE filler during late attention, head-pair 1 at the end,
   summed in SBUF.
 - exp outputs fp16 directly (with a -4 global shift so e^score stays in
   range; the shift cancels in the softmax ratio), halving e-tile traffic.
Softmax skips max-subtraction and gets its denominator for free from an
appended ones-column on V.
"""
import numpy as np

import concourse.bass as bass
import concourse.tile as tile
from concourse import bacc, mybir
from concourse.bass_utils import run_bass_kernel_spmd

F32 = mybir.dt.float32
F32R = mybir.dt.float32r
F16 = mybir.dt.float16

H, NH, HD = 1024, 16, 64
B, S = 2, 2048
P = 128
NCORES = 8
NHL = 4          # heads per core
DQ = NHL * HD    # 256 projection cols per core
NHT = H // P     # 8 h-tiles
NST = S // P     # 16 t-tiles (also s-tiles)
SB = 512         # matmul free-dim block
SS = 1024        # attention s-superblock (2 PSUM banks)
NSB = S // SB    # 4
NSS = S // SS    # 2

EXP_SHIFT = -4.0  # global exp shift (cancels in softmax); keeps e^score
                  # well inside fp16 range for scores up to ~14


def _reshape_free(ap, dims):
    """Reinterpret a contiguous free region of `ap` as `dims`."""
    total = 1
    new = []
    for d in reversed(dims):
        new.append([total, d])
        total *= d
    assert total == ap.free_size()
    return bass.AP(tensor=ap.tensor, offset=ap.offset,
                   ap=[ap.ap[0]] + list(reversed(new)))


def build_program(repeat=1):
    nc = bacc.Bacc("TRN2", target_bir_lowering=False, debug=False,
                   num_devices=NCORES)
    _lp = nc.allow_low_precision(reason="fp16 attention pipeline")
    _lp.__enter__()

    xt_d = nc.dram_tensor("xt", [H, S], F16, kind="ExternalInput").ap()
    wq_d = nc.dram_tensor("wq", [H, DQ], F16, kind="ExternalInput").ap()
    wk_d = nc.dram_tensor("wk", [H, DQ], F16, kind="ExternalInput").ap()
    wv_d = nc.dram_tensor("wv", [H, DQ], F16, kind="ExternalInput").ap()
    wo_d = nc.dram_tensor("wo", [P, 2, H], F32R, kind="ExternalInput").ap()
    bq_d = nc.dram_tensor("bq", [P, 2], F32, kind="ExternalInput").ap()
    bk_d = nc.dram_tensor("bk", [P, 2], F32, kind="ExternalInput").ap()
    bvb_d = nc.dram_tensor("bvb", [P, DQ], F32, kind="ExternalInput").ap()
    mb_d = nc.dram_tensor("maskb", [P, NST], F32, kind="ExternalInput").ap()
    part_d = nc.dram_tensor("part", [S, H], F16, kind="ExternalOutput").ap()

    with tile.TileContext(nc) as tc:
        with tc.tile_pool(name="big", bufs=1) as big, \
             tc.tile_pool(name="consts", bufs=1) as consts, \
             tc.tile_pool(name="epool", bufs=3) as epool, \
             tc.tile_pool(name="bcpool", bufs=2) as bcpool, \
             tc.tile_pool(name="opool", bufs=1) as opool, \
             tc.tile_pool(name="dpool", bufs=2) as dpool, \
             tc.tile_pool(name="ps_sc", bufs=2, space="PSUM") as ps_sc, \
             tc.tile_pool(name="ps_ctx", bufs=1, space="PSUM") as ps_ctx, \
             tc.tile_pool(name="ps_mm", bufs=2, space="PSUM") as ps_mm:

            for _it in range(repeat):
                # ---------------- input loads ----------------
                xt_sb = big.tile([P, NHT, S], F16, tag="xt", name="xt_sb")
                xt_r = xt_d.rearrange("(n p) s -> n p s", p=P)
                wq_sb = consts.tile([P, NHT, DQ], F16, tag="wq", name="wq_sb")
                wk_sb = consts.tile([P, NHT, DQ], F16, tag="wk", name="wk_sb")
                wv_sb = consts.tile([P, NHT, DQ], F16, tag="wv", name="wv_sb")

                xt_rp = xt_d.rearrange("(n p) s -> p n s", p=P)

                def load_x_cols(c0, c1):
                    nc.sync.dma_start(
                        out=xt_sb[:, :, c0:c1], in_=xt_rp[:, :, c0:c1])

                def load_w(w_sb, w_d):
                    nc.sync.dma_start(
                        out=w_sb, in_=w_d.rearrange("(n p) d -> p n d", p=P))

                load_w(wq_sb, wq_d)
                load_w(wk_sb, wk_d)
                load_x_cols(0, 256)
                load_x_cols(256, 512)
                load_w(wv_sb, wv_d)
                load_x_cols(512, 1024)
                load_x_cols(1024, 1536)
                load_x_cols(1536, 2048)

                bq_sb = consts.tile([P, 2], F32, tag="bq", name="bq_sb")
                bk_sb = consts.tile([P, 2], F32, tag="bk", name="bk_sb")
                nc.sync.dma_start(out=bq_sb, in_=bq_d)
                nc.sync.dma_start(out=bk_sb, in_=bk_d)
                bvb_sb = consts.tile([P, DQ], F32, tag="bvb", name="bvb_sb")
                nc.sync.dma_start(out=bvb_sb, in_=bvb_d)
                mb_sb = consts.tile([P, NST], F32, tag="mb", name="mb_sb")
                nc.sync.dma_start(out=mb_sb, in_=mb_d)
                wo_sb = consts.tile([P, 2, H], F32R, tag="wo", name="wo_sb")
                nc.sync.dma_start(out=wo_sb, in_=wo_d)

                # projection outputs: Q^T/K^T in [dv(2 heads), pair, s]
                qT = big.tile([P, 2, S], F16, tag="qT", name="qT")
                kT = big.tile([P, 2, S], F16, tag="kT", name="kT")
                # V (+ones col) in [t, st, head, dv] layout
                vaug = big.tile([P, NST, NHL, HD + 1], F16, tag="vaug",
                                name="vaug")
                nc.vector.memset(vaug[:, :, :, HD:HD + 1], 1.0)

                ctx2 = [big.tile([P, S], F32R, tag=f"ctx2_{pr}",
                                 name=f"ctx2_{pr}") for pr in range(2)]

                rec_rows = {}
                ones128 = consts.tile([1, P], F32R, tag="ones128",
                                      name="ones128")
                one = nc.const_aps.aps[(F32, 1.0)]
                ones_src = bass.AP(tensor=one.tensor, offset=one.offset,
                                   ap=[[one.ap[0][0], 1], [0, P]])
                nc.vector.tensor_copy(ones128, ones_src)

                # ---------------- projection tasks ----------------
                # emitted as single-matmul sub-tasks (~0.2us each) so filler
                # pops never stall the exp-paced attention pipeline
                def qk_subs(dqt, projs="qk", sbs=tuple(range(NSB))):
                    sel = {"q": (wq_sb, bq_sb, qT, "q"),
                           "k": (wk_sb, bk_sb, kT, "k")}
                    subs = []
                    for sb_i in sbs:
                        for w_sb, b_sb, out_sb, nm in (sel[p] for p in projs):
                            st8 = {}

                            def mm(ht, w_sb=w_sb, sb_i=sb_i, st8=st8, nm=nm):
                                def t():
                                    if ht == 0:
                                        st8["acc"] = ps_mm.tile(
                                            [P, SB], F32, tag="mm512",
                                            name=f"acc_{nm}{dqt}_{sb_i}")
                                    nc.tensor.matmul(
                                        st8["acc"],
                                        w_sb[:, ht, dqt * P:(dqt + 1) * P],
                                        xt_sb[:, ht,
                                              sb_i * SB:(sb_i + 1) * SB],
                                        start=(ht == 0), stop=(ht == NHT - 1))
                                return t

                            def drain(b_sb=b_sb, out_sb=out_sb, sb_i=sb_i,
                                      st8=st8):
                                def t():
                                    nc.vector.tensor_scalar_add(
                                        out_sb[:, dqt,
                                               sb_i * SB:(sb_i + 1) * SB],
                                        st8["acc"], b_sb[:, dqt:dqt + 1])
                                return t

                            subs += [mm(ht) for ht in range(NHT)]
                            subs.append(drain())
                    return subs

                def v_subs(dqt, sts=tuple(range(NST))):
                    subs = []
                    for st in sts:
                        st8 = {}

                        def mm(ht, st=st, st8=st8):
                            def t():
                                if ht == 0:
                                    st8["acc"] = ps_mm.tile(
                                        [P, SB], F32, tag="mm512",
                                        name=f"vacc{dqt}_{st}")
                                nc.tensor.matmul(
                                    st8["acc"][:, 0:P],
                                    xt_sb[:, ht, st * P:(st + 1) * P],
                                    wv_sb[:, ht, dqt * P:(dqt + 1) * P],
                                    start=(ht == 0), stop=(ht == NHT - 1))
                            return t

                        def drain(st=st, st8=st8):
                            def t():
                                nc.vector.tensor_add(
                                    vaug[:, st, 2 * dqt:2 * dqt + 2, 0:HD],
                                    _reshape_free(st8["acc"][:, 0:P], [2, HD]),
                                    _reshape_free(
                                        bvb_sb[:, dqt * P:(dqt + 1) * P],
                                        [2, HD]))
                            return t

                        subs += [mm(ht) for ht in range(NHT)]
                        subs.append(drain())
                    return subs

                # ---------------- attention ----------------
                def attention(h, filler, rate=2.0, mid=None):
                    base = HD * (h % 2)
                    dvt = h // 2
                    pr = h // 2
                    row = HD * (h % 2)
                    budget = 0.0
                    rates = rate if isinstance(rate, tuple) else (rate, rate)
                    for ssb in range(NSS):
                        rate = rates[ssb]
                        if ssb == 1 and mid is not None:
                            mid()
                        acc = ps_ctx.tile([HD + 1, SS], F32, tag="ctxps",
                                          name=f"ctx_{h}_{ssb}")
                        prev_e = None
                        # ctx runs one t-tile behind exp so the PE (in-order)
                        # never waits on the ACT exp latency
                        for tt in range(NST + 1):
                            budget += rate
                            while filler and budget >= 1.0:
                                filler.pop(0)()
                                budget -= 1.0
                            if tt < NST:
                                sc = ps_sc.tile([P, SS], F32, tag="sc",
                                                name=f"sc_{h}_{ssb}_{tt}")
                                for half in range(2):
                                    sb_i = 2 * ssb + half
                                    nc.tensor.matmul(
                                        sc[:, half * SB:(half + 1) * SB],
                                        kT[base:base + HD, dvt,
                                           tt * P:(tt + 1) * P],
                                        qT[base:base + HD, dvt,
                                           sb_i * SB:(sb_i + 1) * SB],
                                        start=True, stop=True)
                            if tt > 0:
                                for half in range(2):
                                    nc.tensor.matmul(
                                        acc[:, half * SB:(half + 1) * SB],
                                        vaug[:, tt - 1, h, :],
                                        prev_e[:, half * SB:(half + 1) * SB],
                                        start=(tt == 1), stop=(tt == NST))
                            if tt < NST:
                                e = epool.tile([P, SS], F16, tag="e",
                                               name=f"e_{h}_{ssb}_{tt}")
                                nc.scalar.activation(
                                    out=e, in_=sc,
                                    func=mybir.ActivationFunctionType.Exp,
                                    bias=mb_sb[:, tt:tt + 1], scale=1.0 / 8.0)
                                prev_e = e
                        # drain ctx + denominator
                        for half in range(2):
                            sb_i = 2 * ssb + half
                            nc.vector.tensor_copy(
                                ctx2[pr][row:row + HD,
                                         sb_i * SB:(sb_i + 1) * SB],
                                acc[0:HD, half * SB:(half + 1) * SB])
                        # reciprocal of the denominator row, in place on-chip
                        rec_row = dpool.tile([1, SS], F32R, tag="recrow",
                                             name=f"recrow_{h}_{ssb}")
                        nc.vector.reciprocal(rec_row, acc[HD:HD + 1, :])
                        rec_rows[(h, ssb)] = rec_row

                def rec_chain(h, ssbs=(0, 1)):
                    # broadcast 1/den over the dv rows with a K=1 PE outer
                    # product (ones64 x rec_row) and scale ctx2 in place --
                    # fully on-chip, no DRAM round trip
                    pr = h // 2
                    row = HD * (h % 2)
                    for ssb in ssbs:
                        rr = rec_rows[(h, ssb)]
                        for half in range(2):
                            sb_i = 2 * ssb + half
                            bc = ps_mm.tile([P, SB], F32, tag="mm512",
                                            name=f"bc_{h}_{sb_i}")
                            nc.tensor.matmul(
                                bc, ones128,
                                rr[:, half * SB:(half + 1) * SB],
                                start=True, stop=True)
                            nc.vector.tensor_mul(
                                ctx2[pr][row:row + HD,
                                         sb_i * SB:(sb_i + 1) * SB],
                                ctx2[pr][row:row + HD,
                                         sb_i * SB:(sb_i + 1) * SB],
                                bc[row:row + HD, :])

                # ---------------- output projection ----------------
                o_st = [None] * NST

                def outproj_p0(st, j):
                    def t():
                        if j == 0:
                            o_st[st] = opool.tile([P, H], F16, tag=f"o_{st}",
                                                  name=f"o_{st}")
                        o = o_st[st]
                        po = ps_mm.tile([P, SB], F32, tag="mm512",
                                        name=f"po0_{st}_{j}")
                        nc.tensor.matmul(
                            po,
                            ctx2[0][:, st * P:(st + 1) * P],
                            wo_sb[:, 0, j * SB:(j + 1) * SB],
                            start=True, stop=True)
                        nc.vector.tensor_copy(o[:, j * SB:(j + 1) * SB], po)
                    return t

                def outproj_p1(st):
                    def t():
                        o = o_st[st]
                        for j in range(2):
                            po = ps_mm.tile([P, SB], F32, tag="mm512",
                                            name=f"po1_{st}_{j}")
                            nc.tensor.matmul(
                                po,
                                ctx2[1][:, st * P:(st + 1) * P],
                                wo_sb[:, 1, j * SB:(j + 1) * SB],
                                start=True, stop=True)
                            nc.vector.tensor_add(
                                o[:, j * SB:(j + 1) * SB],
                                po, o[:, j * SB:(j + 1) * SB])
                        nc.sync.dma_start(
                            out=part_d[st * P:(st + 1) * P, :], in_=o)
                    return t

                def outproj(st, use_act):
                    # single pass over both head pairs; at the kernel tail
                    # the drains alternate DVE / ACT so neither paces it
                    def t():
                        o = opool.tile([P, H], F16, tag=f"o_{st}",
                                       name=f"o_{st}")
                        for j in range(2):
                            po = ps_mm.tile([P, SB], F32, tag="mm512",
                                            name=f"po_{st}_{j}")
                            for pr in range(2):
                                nc.tensor.matmul(
                                    po,
                                    ctx2[pr][:, st * P:(st + 1) * P],
                                    wo_sb[:, pr, j * SB:(j + 1) * SB],
                                    start=(pr == 0), stop=(pr == 1))
                            if use_act and j % 2 == 1:
                                nc.scalar.copy(o[:, j * SB:(j + 1) * SB], po)
                            else:
                                nc.vector.tensor_copy(
                                    o[:, j * SB:(j + 1) * SB], po)
                        nc.sync.dma_start(
                            out=part_d[st * P:(st + 1) * P, :], in_=o)
                    return t

                # ---------------- schedule ----------------
                # inline lead: only what h0's first steps strictly need
                # (K0/Q0 for s,t < 512-1024, V pair-0 tiles 0-3); the rest
                # drips as deadline-ordered fillers during h0-ssb0
                for t in (qk_subs(0, "kq", (0,)) + qk_subs(0, "q", (1,))
                          + v_subs(0, (0, 1, 2, 3))):
                    t()
                # deadline-ordered h0-ssb0 fillers at 9 pops/step: K0-sb_i
                # EMITTED by step 4i, v0_st by step st (emission order is
                # what guarantees readers see written tiles)
                fill = (qk_subs(0, "k", (1,)) + v_subs(0, (4, 5))
                        + qk_subs(0, "k", (2,)) + v_subs(0, (6, 7, 8))
                        + qk_subs(0, "k", (3,))
                        + v_subs(0, (9, 10, 11, 12, 13, 14, 15))
                        + qk_subs(0, "q", (2, 3))
                        + v_subs(1)
                        + qk_subs(1, "k") + qk_subs(1, "q", (0, 1)))
                attention(0, fill, rate=(9.0, 3.2))
                rec_chain(0)
                attention(1, fill, rate=3.2)
                while fill:
                    fill.pop(0)()
                rec_chain(1)
                fill2 = qk_subs(1, "q", (2, 3)) + [
                    outproj_p0(st, j) for st in range(NST // 2)
                    for j in range(2)]
                attention(2, fill2, rate=1.2)
                rec_chain(2)

                def h3_mid():
                    # after h3's first superblock: normalize its s<1024 rows,
                    # then finish the first-half output projection as filler
                    while fill2:
                        fill2.pop(0)()
                    rec_chain(3, ssbs=(0,))
                    fill2.extend(outproj_p1(st) for st in range(NST // 2))

                attention(3, fill2, rate=1.0, mid=h3_mid)
                while fill2:
                    fill2.pop(0)()
                rec_chain(3, ssbs=(1,))
                for st in range(NST // 2, NST):
                    outproj(st, True)()

    nc.compile()
    return nc


_CACHE = {}


def _get_program(repeat=1):
    key = repeat
    if key not in _CACHE:
        _CACHE[key] = build_program(repeat)
    return _CACHE[key]


def _make_in_maps(inputs):
    X = np.asarray(inputs["X"], dtype=np.float32)
    mask = np.asarray(inputs["mask"], dtype=np.float32)
    Wq = np.asarray(inputs["Wq"], dtype=np.float32)
    Wk = np.asarray(inputs["Wk"], dtype=np.float32)
    Wv = np.asarray(inputs["Wv"], dtype=np.float32)
    Wo = np.asarray(inputs["Wo"], dtype=np.float32)
    bq = np.asarray(inputs["bq"], dtype=np.float32)
    bk = np.asarray(inputs["bk"], dtype=np.float32)
    bv = np.asarray(inputs["bv"], dtype=np.float32)

    f16 = np.float16
    in_maps = []
    xts = [np.ascontiguousarray(X[b].T).astype(f16) for b in range(B)]
    maskbs = [np.ascontiguousarray(-1e6 * (1.0 - mask[b])) for b in range(B)]
    for c in range(NCORES):
        b = c // 4
        g = c % 4
        cols = slice(g * DQ, (g + 1) * DQ)
        mb2 = (maskbs[b].reshape(NST, P).T + EXP_SHIFT).astype(np.float32)
        wo2 = Wo[cols, :].reshape(2, P, H).transpose(1, 0, 2)
        in_maps.append({
            "xt": xts[b],
            "wq": np.ascontiguousarray(Wq[:, cols]).astype(f16),
            "wk": np.ascontiguousarray(Wk[:, cols]).astype(f16),
            "wv": np.ascontiguousarray(Wv[:, cols]).astype(f16),
            "wo": np.ascontiguousarray(wo2),
            "bq": np.ascontiguousarray(bq[cols].reshape(2, P).T),
            "bk": np.ascontiguousarray(bk[cols].reshape(2, P).T),
            "bvb": np.ascontiguousarray(
                np.tile(bv[cols].reshape(1, DQ), (P, 1))).astype(np.float32),
            "maskb": np.ascontiguousarray(mb2),
        })
    return in_maps


def kernel(X, mask, Wq, bq, Wk, bk, Wv, bv, Wo, bo):
    bo = np.asarray(bo, dtype=np.float32)
    nc = _get_program()
    in_maps = _make_in_maps(dict(X=X, mask=mask, Wq=Wq, bq=bq, Wk=Wk, bk=bk,
                                 Wv=Wv, bv=bv, Wo=Wo, bo=bo))
    res = run_bass_kernel_spmd(nc, in_maps, list(range(NCORES))).results
    out = np.zeros((B, S, H), dtype=np.float32)
    for c in range(NCORES):
        out[c // 4] += res[c]["part"]
    out += bo
    return out


# revision 62
# speedup vs baseline: 19.6574x; 1.0404x over previous
"""Multi-head attention (B=2, S=2048, H=1024, 16 heads x 64) on 8 NeuronCores.

Sharding: tensor-parallel over heads x data-parallel over batch.
Core c handles batch (c // 4) and heads [4*(c%4), 4*(c%4)+4).
Each core computes its 4 heads' QKV projections, attention, and the partial
output projection ctx_h @ Wo_h; the host sums the 4 partials per batch.

The datapath is fp16 (noise ~5e-4; fp8 was tried and its ~2.5%/stage
quantization noise transfers 1:1 through the softmax-weighted mean, far
over the accuracy budget). fp16 matmuls run at the same 1 cycle/row as
fp32r but with half the SBUF/DMA traffic. Structural savings vs the fp32
baseline:
 - V is computed directly in [t, dv] layout by making X the stationary
   matmul operand, eliminating all PE transposes and their drains.
 - The output projection packs the two heads of a pair on the contraction
   dim (K=128 instead of 64), halving its PE time. It runs in two passes:
   head-pair 0 as PE filler during late attention, head-pair 1 at the end,
   summed in SBUF.
 - exp outputs fp16 directly (with a -4 global shift so e^score stays in
   range; the shift cancels in the softmax ratio), halving e-tile traffic.
Softmax skips max-subtraction and gets its denominator for free from an
appended ones-column on V.
"""
import numpy as np

import concourse.bass as bass
import concourse.tile as tile
from concourse import bacc, mybir
from concourse.bass_utils import run_bass_kernel_spmd

F32 = mybir.dt.float32
F32R = mybir.dt.float32r
F16 = mybir.dt.float16

H, NH, HD = 1024, 16, 64
B, S = 2, 2048
P = 128
NCORES = 8
NHL = 4          # heads per core
DQ = NHL * HD    # 256 projection cols per core
NHT = H // P     # 8 h-tiles
NST = S // P     # 16 t-tiles (also s-tiles)
SB = 512         # matmul free-dim block
SS = 1024        # attention s-superblock (2 PSUM banks)
NSB = S // SB    # 4
NSS = S // SS    # 2

EXP_SHIFT = -4.0  # global exp shift (cancels in softmax); keeps e^score
                  # well inside fp16 range for scores up to ~14


def _reshape_free(ap, dims):
    """Reinterpret a contiguous free region of `ap` as `dims`."""
    total = 1
    new = []
    for d in reversed(dims):
        new.append([total, d])
        total *= d
    assert total == ap.free_size()
    return bass.AP(tensor=ap.tensor, offset=ap.offset,
                   ap=[ap.ap[0]] + list(reversed(new)))


def build_program(repeat=1):
    nc = bacc.Bacc("TRN2", target_bir_lowering=False, debug=False,
                   num_devices=NCORES)
    _lp = nc.allow_low_precision(reason="fp16 attention pipeline")
    _lp.__enter__()

    xt_d = nc.dram_tensor("xt", [H, S], F16, kind="ExternalInput").ap()
    wq_d = nc.dram_tensor("wq", [H, DQ], F16, kind="ExternalInput").ap()
    wk_d = nc.dram_tensor("wk", [H, DQ], F16, kind="ExternalInput").ap()
    wv_d = nc.dram_tensor("wv", [H, DQ], F16, kind="ExternalInput").ap()
    wo_d = nc.dram_tensor("wo", [P, 2, H], F32R, kind="ExternalInput").ap()
    bq_d = nc.dram_tensor("bq", [P, 2], F32, kind="ExternalInput").ap()
    bk_d = nc.dram_tensor("bk", [P, 2], F32, kind="ExternalInput").ap()
    bvb_d = nc.dram_tensor("bvb", [P, DQ], F32, kind="ExternalInput").ap()
    mb_d = nc.dram_tensor("maskb", [P, NST], F32, kind="ExternalInput").ap()
    part_d = nc.dram_tensor("part", [S, H], F16, kind="ExternalOutput").ap()

    with tile.TileContext(nc) as tc:
        with tc.tile_pool(name="big", bufs=1) as big, \
             tc.tile_pool(name="consts", bufs=1) as consts, \
             tc.tile_pool(name="epool", bufs=5) as epool, \
             tc.tile_pool(name="bcpool", bufs=2) as bcpool, \
             tc.tile_pool(name="opool", bufs=1) as opool, \
             tc.tile_pool(name="dpool", bufs=2) as dpool, \
             tc.tile_pool(name="ps_sc", bufs=2, space="PSUM") as ps_sc, \
             tc.tile_pool(name="ps_ctx", bufs=1, space="PSUM") as ps_ctx, \
             tc.tile_pool(name="ps_mm", bufs=2, space="PSUM") as ps_mm:

            for _it in range(repeat):
                # ---------------- input loads ----------------
                xt_sb = big.tile([P, NHT, S], F16, tag="xt", name="xt_sb")
                xt_r = xt_d.rearrange("(n p) s -> n p s", p=P)
                wq_sb = consts.tile([P, NHT, DQ], F16, tag="wq", name="wq_sb")
                wk_sb = consts.tile([P, NHT, DQ], F16, tag="wk", name="wk_sb")
                wv_sb = consts.tile([P, NHT, DQ], F16, tag="wv", name="wv_sb")

                xt_rp = xt_d.rearrange("(n p) s -> p n s", p=P)

                def load_x_cols(c0, c1):
                    nc.sync.dma_start(
                        out=xt_sb[:, :, c0:c1], in_=xt_rp[:, :, c0:c1])

                def load_w(w_sb, w_d):
                    nc.sync.dma_start(
                        out=w_sb, in_=w_d.rearrange("(n p) d -> p n d", p=P))

                load_w(wk_sb, wk_d)
                load_x_cols(0, 256)
                load_w(wq_sb, wq_d)
                load_x_cols(256, 512)
                load_w(wv_sb, wv_d)
                load_x_cols(512, 1024)
                load_x_cols(1024, 1536)
                load_x_cols(1536, 2048)

                bq_sb = consts.tile([P, 2], F32, tag="bq", name="bq_sb")
                bk_sb = consts.tile([P, 2], F32, tag="bk", name="bk_sb")
                nc.sync.dma_start(out=bq_sb, in_=bq_d)
                nc.sync.dma_start(out=bk_sb, in_=bk_d)
                bvb_sb = consts.tile([P, DQ], F32, tag="bvb", name="bvb_sb")
                nc.sync.dma_start(out=bvb_sb, in_=bvb_d)
                mb_sb = consts.tile([P, NST], F32, tag="mb", name="mb_sb")
                nc.sync.dma_start(out=mb_sb, in_=mb_d)
                wo_sb = consts.tile([P, 2, H], F32R, tag="wo", name="wo_sb")
                nc.sync.dma_start(out=wo_sb, in_=wo_d)

                # projection outputs: Q^T/K^T in [dv(2 heads), pair, s]
                qT = big.tile([P, 2, S], F16, tag="qT", name="qT")
                kT = big.tile([P, 2, S], F16, tag="kT", name="kT")
                # V (+ones col) in [t, st, head, dv] layout
                vaug = big.tile([P, NST, NHL, HD + 1], F16, tag="vaug",
                                name="vaug")
                nc.vector.memset(vaug[:, :, :, HD:HD + 1], 1.0)

                ctx2 = [big.tile([P, S], F32R, tag=f"ctx2_{pr}",
                                 name=f"ctx2_{pr}") for pr in range(2)]

                rec_rows = {}
                ones128 = consts.tile([1, P], F32R, tag="ones128",
                                      name="ones128")
                one = nc.const_aps.aps[(F32, 1.0)]
                ones_src = bass.AP(tensor=one.tensor, offset=one.offset,
                                   ap=[[one.ap[0][0], 1], [0, P]])
                nc.vector.tensor_copy(ones128, ones_src)

                # ---------------- projection tasks ----------------
                # emitted as single-matmul sub-tasks (~0.2us each) so filler
                # pops never stall the exp-paced attention pipeline
                def qk_subs(dqt, projs="qk", sbs=tuple(range(NSB))):
                    sel = {"q": (wq_sb, bq_sb, qT, "q"),
                           "k": (wk_sb, bk_sb, kT, "k")}
                    subs = []
                    for sb_i in sbs:
                        for w_sb, b_sb, out_sb, nm in (sel[p] for p in projs):
                            st8 = {}

                            def mm(ht, w_sb=w_sb, sb_i=sb_i, st8=st8, nm=nm):
                                def t():
                                    if ht == 0:
                                        st8["acc"] = ps_mm.tile(
                                            [P, SB], F32, tag="mm512",
                                            name=f"acc_{nm}{dqt}_{sb_i}")
                                    nc.tensor.matmul(
                                        st8["acc"],
                                        w_sb[:, ht, dqt * P:(dqt + 1) * P],
                                        xt_sb[:, ht,
                                              sb_i * SB:(sb_i + 1) * SB],
                                        start=(ht == 0), stop=(ht == NHT - 1))
                                return t

                            def drain(b_sb=b_sb, out_sb=out_sb, sb_i=sb_i,
                                      st8=st8):
                                def t():
                                    nc.vector.tensor_scalar_add(
                                        out_sb[:, dqt,
                                               sb_i * SB:(sb_i + 1) * SB],
                                        st8["acc"], b_sb[:, dqt:dqt + 1])
                                return t

                            subs += [mm(ht) for ht in range(NHT)]
                            subs.append(drain())
                    return subs

                def v_subs(dqt, sts=tuple(range(NST))):
                    subs = []
                    for st in sts:
                        st8 = {}

                        def mm(ht, st=st, st8=st8):
                            def t():
                                if ht == 0:
                                    st8["acc"] = ps_mm.tile(
                                        [P, SB], F32, tag="mm512",
                                        name=f"vacc{dqt}_{st}")
                                nc.tensor.matmul(
                                    st8["acc"][:, 0:P],
                                    xt_sb[:, ht, st * P:(st + 1) * P],
                                    wv_sb[:, ht, dqt * P:(dqt + 1) * P],
                                    start=(ht == 0), stop=(ht == NHT - 1))
                            return t

                        def drain(st=st, st8=st8):
                            def t():
                                nc.vector.tensor_add(
                                    vaug[:, st, 2 * dqt:2 * dqt + 2, 0:HD],
                                    _reshape_free(st8["acc"][:, 0:P], [2, HD]),
                                    _reshape_free(
                                        bvb_sb[:, dqt * P:(dqt + 1) * P],
                                        [2, HD]))
                            return t

                        subs += [mm(ht) for ht in range(NHT)]
                        subs.append(drain())
                    return subs

                # ---------------- attention ----------------
                def attention(h, filler, rate=2.0, mid=None):
                    base = HD * (h % 2)
                    dvt = h // 2
                    pr = h // 2
                    row = HD * (h % 2)
                    budget = 0.0
                    rates = rate if isinstance(rate, tuple) else (rate, rate)
                    for ssb in range(NSS):
                        rate = rates[ssb]
                        if ssb == 1 and mid is not None:
                            mid()
                        acc = ps_ctx.tile([HD + 1, SS], F32, tag="ctxps",
                                          name=f"ctx_{h}_{ssb}")
                        es = {}
                        # ctx runs TWO t-tiles behind exp so the PE (in-order)
                        # never waits on the ACT exp latency or its semaphore
                        LAG = 3
                        for tt in range(NST + LAG):
                            budget += rate
                            while filler and budget >= 1.0:
                                filler.pop(0)()
                                budget -= 1.0
                            if tt < NST:
                                sc = ps_sc.tile([P, SS], F32, tag="sc",
                                                name=f"sc_{h}_{ssb}_{tt}")
                                for half in range(2):
                                    sb_i = 2 * ssb + half
                                    nc.tensor.matmul(
                                        sc[:, half * SB:(half + 1) * SB],
                                        kT[base:base + HD, dvt,
                                           tt * P:(tt + 1) * P],
                                        qT[base:base + HD, dvt,
                                           sb_i * SB:(sb_i + 1) * SB],
                                        start=True, stop=True)
                            if tt >= LAG:
                                e_in = es.pop(tt - LAG)
                                for half in range(2):
                                    nc.tensor.matmul(
                                        acc[:, half * SB:(half + 1) * SB],
                                        vaug[:, tt - LAG, h, :],
                                        e_in[:, half * SB:(half + 1) * SB],
                                        start=(tt == LAG),
                                        stop=(tt == NST + LAG - 1))
                            if tt < NST:
                                e = epool.tile([P, SS], F16, tag="e",
                                               name=f"e_{h}_{ssb}_{tt}")
                                nc.scalar.activation(
                                    out=e, in_=sc,
                                    func=mybir.ActivationFunctionType.Exp,
                                    bias=mb_sb[:, tt:tt + 1], scale=1.0 / 8.0)
                                es[tt] = e
                        # drain ctx + denominator
                        for half in range(2):
                            sb_i = 2 * ssb + half
                            nc.vector.tensor_copy(
                                ctx2[pr][row:row + HD,
                                         sb_i * SB:(sb_i + 1) * SB],
                                acc[0:HD, half * SB:(half + 1) * SB])
                        # reciprocal of the denominator row, in place on-chip
                        rec_row = dpool.tile([1, SS], F32R, tag="recrow",
                                             name=f"recrow_{h}_{ssb}")
                        nc.vector.reciprocal(rec_row, acc[HD:HD + 1, :])
                        rec_rows[(h, ssb)] = rec_row

                def rec_chain(h, ssbs=(0, 1)):
                    # broadcast 1/den over the dv rows with a K=1 PE outer
                    # product (ones64 x rec_row) and scale ctx2 in place --
                    # fully on-chip, no DRAM round trip
                    pr = h // 2
                    row = HD * (h % 2)
                    for ssb in ssbs:
                        rr = rec_rows[(h, ssb)]
                        for half in range(2):
                            sb_i = 2 * ssb + half
                            bc = ps_mm.tile([P, SB], F32, tag="mm512",
                                            name=f"bc_{h}_{sb_i}")
                            nc.tensor.matmul(
                                bc, ones128,
                                rr[:, half * SB:(half + 1) * SB],
                                start=True, stop=True)
                            nc.vector.tensor_mul(
                                ctx2[pr][row:row + HD,
                                         sb_i * SB:(sb_i + 1) * SB],
                                ctx2[pr][row:row + HD,
                                         sb_i * SB:(sb_i + 1) * SB],
                                bc[row:row + HD, :])

                # ---------------- output projection ----------------
                o_st = [None] * NST

                def outproj_p0(st, j):
                    def t():
                        if j == 0:
                            o_st[st] = opool.tile([P, H], F16, tag=f"o_{st}",
                                                  name=f"o_{st}")
                        o = o_st[st]
                        po = ps_mm.tile([P, SB], F32, tag="mm512",
                                        name=f"po0_{st}_{j}")
                        nc.tensor.matmul(
                            po,
                            ctx2[0][:, st * P:(st + 1) * P],
                            wo_sb[:, 0, j * SB:(j + 1) * SB],
                            start=True, stop=True)
                        nc.vector.tensor_copy(o[:, j * SB:(j + 1) * SB], po)
                    return t

                def outproj_p1(st):
                    def t():
                        o = o_st[st]
                        for j in range(2):
                            po = ps_mm.tile([P, SB], F32, tag="mm512",
                                            name=f"po1_{st}_{j}")
                            nc.tensor.matmul(
                                po,
                                ctx2[1][:, st * P:(st + 1) * P],
                                wo_sb[:, 1, j * SB:(j + 1) * SB],
                                start=True, stop=True)
                            nc.vector.tensor_add(
                                o[:, j * SB:(j + 1) * SB],
                                po, o[:, j * SB:(j + 1) * SB])
                        nc.sync.dma_start(
                            out=part_d[st * P:(st + 1) * P, :], in_=o)
                    return t

                def outproj(st, use_act):
                    # single pass over both head pairs; at the kernel tail
                    # the drains alternate DVE / ACT so neither paces it
                    def t():
                        o = opool.tile([P, H], F16, tag=f"o_{st}",
                                       name=f"o_{st}")
                        for j in range(2):
                            po = ps_mm.tile([P, SB], F32, tag="mm512",
                                            name=f"po_{st}_{j}")
                            for pr in range(2):
                                nc.tensor.matmul(
                                    po,
                                    ctx2[pr][:, st * P:(st + 1) * P],
                                    wo_sb[:, pr, j * SB:(j + 1) * SB],
                                    start=(pr == 0), stop=(pr == 1))
                            if use_act and j % 2 == 1:
                                nc.scalar.copy(o[:, j * SB:(j + 1) * SB], po)
                            else:
                                nc.vector.tensor_copy(
                                    o[:, j * SB:(j + 1) * SB], po)
                        nc.sync.dma_start(
                            out=part_d[st * P:(st + 1) * P, :], in_=o)
                    return t

                # ---------------- schedule ----------------
                # inline lead: only what h0's first steps strictly need
                # (K0/Q0 for s,t < 512-1024, V pair-0 tiles 0-3); the rest
                # drips as deadline-ordered fillers during h0-ssb0
                for t in (qk_subs(0, "kq", (0,)) + qk_subs(0, "q", (1,))
                          + v_subs(0, (0, 1, 2, 3))):
                    t()
                # deadline-ordered h0-ssb0 fillers at 9 pops/step: K0-sb_i
                # EMITTED by step 4i, v0_st by step st (emission order is
                # what guarantees readers see written tiles)
                fill = (qk_subs(0, "k", (1,)) + v_subs(0, (4, 5))
                        + qk_subs(0, "k", (2,)) + v_subs(0, (6, 7, 8))
                        + qk_subs(0, "k", (3,))
                        + v_subs(0, (9, 10, 11, 12, 13, 14, 15))
                        + qk_subs(0, "q", (2, 3))
                        + v_subs(1)
                        + qk_subs(1, "k") + qk_subs(1, "q", (0, 1)))
                attention(0, fill, rate=(9.0, 3.2))
                rec_chain(0)
                attention(1, fill, rate=3.2)
                while fill:
                    fill.pop(0)()
                rec_chain(1)
                fill2 = qk_subs(1, "q", (2, 3)) + [
                    outproj_p0(st, j) for st in range(NST // 2)
                    for j in range(2)]
                attention(2, fill2, rate=1.2)
                rec_chain(2)

                def h3_mid():
                    # after h3's first superblock: normalize its s<1024 rows,
                    # then finish the first-half output projection as filler
                    while fill2:
                        fill2.pop(0)()
                    rec_chain(3, ssbs=(0,))
                    fill2.extend(outproj_p1(st) for st in range(NST // 2))

                attention(3, fill2, rate=1.0, mid=h3_mid)
                while fill2:
                    fill2.pop(0)()
                rec_chain(3, ssbs=(1,))
                for st in range(NST // 2, NST):
                    outproj(st, True)()

    nc.compile()
    return nc


_CACHE = {}


def _get_program(repeat=1):
    key = repeat
    if key not in _CACHE:
        _CACHE[key] = build_program(repeat)
    return _CACHE[key]


def _make_in_maps(inputs):
    X = np.asarray(inputs["X"], dtype=np.float32)
    mask = np.asarray(inputs["mask"], dtype=np.float32)
    Wq = np.asarray(inputs["Wq"], dtype=np.float32)
    Wk = np.asarray(inputs["Wk"], dtype=np.float32)
    Wv = np.asarray(inputs["Wv"], dtype=np.float32)
    Wo = np.asarray(inputs["Wo"], dtype=np.float32)
    bq = np.asarray(inputs["bq"], dtype=np.float32)
    bk = np.asarray(inputs["bk"], dtype=np.float32)
    bv = np.asarray(inputs["bv"], dtype=np.float32)

    f16 = np.float16
    in_maps = []
    xts = [np.ascontiguousarray(X[b].T).astype(f16) for b in range(B)]
    maskbs = [np.ascontiguousarray(-1e6 * (1.0 - mask[b])) for b in range(B)]
    for c in range(NCORES):
        b = c // 4
        g = c % 4
        cols = slice(g * DQ, (g + 1) * DQ)
        mb2 = (maskbs[b].reshape(NST, P).T + EXP_SHIFT).astype(np.float32)
        wo2 = Wo[cols, :].reshape(2, P, H).transpose(1, 0, 2)
        in_maps.append({
            "xt": xts[b],
            "wq": np.ascontiguousarray(Wq[:, cols]).astype(f16),
            "wk": np.ascontiguousarray(Wk[:, cols]).astype(f16),
            "wv": np.ascontiguousarray(Wv[:, cols]).astype(f16),
            "wo": np.ascontiguousarray(wo2),
            "bq": np.ascontiguousarray(bq[cols].reshape(2, P).T),
            "bk": np.ascontiguousarray(bk[cols].reshape(2, P).T),
            "bvb": np.ascontiguousarray(
                np.tile(bv[cols].reshape(1, DQ), (P, 1))).astype(np.float32),
            "maskb": np.ascontiguousarray(mb2),
        })
    return in_maps


def kernel(X, mask, Wq, bq, Wk, bk, Wv, bv, Wo, bo):
    bo = np.asarray(bo, dtype=np.float32)
    nc = _get_program()
    in_maps = _make_in_maps(dict(X=X, mask=mask, Wq=Wq, bq=bq, Wk=Wk, bk=bk,
                                 Wv=Wv, bv=bv, Wo=Wo, bo=bo))
    res = run_bass_kernel_spmd(nc, in_maps, list(range(NCORES))).results
    out = np.zeros((B, S, H), dtype=np.float32)
    for c in range(NCORES):
        out[c // 4] += res[c]["part"]
    out += bo
    return out
